# revision 1
# baseline (speedup 1.0000x reference)
"""Multi-head latent attention (MLA) Bass kernel for 8 Trainium2 NeuronCores.

Sharding: core = (batch b in 0..1, kv-group g in 0..3). Each core computes
batch b, heads 4g..4g+3 (which share kv head g). The latent projection
(x @ Wl) is replicated within a batch's 4 cores; the output projection is
computed as per-core partial sums over the core's 512-column slice of the
concatenated context, summed on the host.

All matmul inputs are bf16 (host-cast) with f32 PSUM accumulation. The host
uploads x pre-transposed (features on partitions) so every matmul operand is
in natural lhsT/rhs layout and no device-side transposes of activations are
needed except the attention-probability transpose, done on the PE.
"""

import numpy as np
import ml_dtypes
from contextlib import ExitStack

B = 2
T = 2048
D_IN = 2048
D_OUT = 2048
N_HEAD = 16
N_KV = 4
HEAD_DIM = 128
KV_DIM = 64
LATENT = 1024
GROUP = N_HEAD // N_KV          # 4
HPC = 4                          # heads per core
QCOLS = HPC * HEAD_DIM           # 512 columns of Wq/rows of Wo per core
P = 128
NKT = D_IN // P                  # 16 contraction tiles over D_IN
LKT = LATENT // P                # 8 contraction tiles over LATENT
NQT = T // 512                   # 4 free-dim tiles of 512
NB = T // P                      # 16 blocks of 128 (q and k)
SCALE = 1.0 / np.sqrt(KV_DIM)
EXP_BIAS = -4.0                  # constant shift inside exp; cancels in softmax

BF16 = ml_dtypes.bfloat16

_PROGRAM_CACHE = {}


def _emit_lat(tc, io):
    """Launch A: latT slice for this core's 256 latent columns (no
    replication — the 4 cores of a batch each produce a disjoint slice)."""
    from concourse import mybir

    nc = tc.nc
    fp32 = mybir.dt.float32
    bf16 = mybir.dt.bfloat16
    AF = mybir.ActivationFunctionType
    xT, wlg, blg, latg = io["xT"], io["wlg"], io["blg"], io["latg"]
    MLOC = LATENT // 4 // P        # 2 m-blocks of 128

    with ExitStack() as ctx:
        ek = ctx.enter_context
        pconst = ek(tc.tile_pool(name="constA", bufs=1))
        blg_sb = pconst.tile([P, MLOC], fp32, tag="blg")
        nc.sync.dma_start(blg_sb[:], blg[:])
        pw = ek(tc.tile_pool(name="wA", bufs=1))
        wl_sb = [pw.tile([P, LATENT // 4], bf16, tag=f"wla{k}", name=f"wla{k}")
                 for k in range(NKT)]
        for k in range(NKT):
            nc.sync.dma_start(wl_sb[k][:], wlg[P * k:P * (k + 1), :])
        px = ek(tc.tile_pool(name="xA", bufs=24))
        ptmp = ek(tc.tile_pool(name="tmpA", bufs=3))
        plat = ek(tc.tile_pool(name="latA", bufs=4))
        pps = ek(tc.tile_pool(name="psA", bufs=3, space="PSUM"))

        for n in range(NQT):
            ns = slice(512 * n, 512 * (n + 1))
            x_n = []
            for k in range(NKT):
                xt = px.tile([P, 512], bf16, tag="x", name="xtA")
                nc.sync.dma_start(xt[:], xT[P * k:P * (k + 1), ns])
                x_n.append(xt)
            for m in range(MLOC):
                ps = pps.tile([P, 512], fp32, tag="ps")
                for k in range(NKT):
                    nc.tensor.matmul(
                        ps[:], wl_sb[k][:, P * m:P * (m + 1)], x_n[k][:],
                        start=(k == 0), stop=(k == NKT - 1))
                zt = ptmp.tile([P, 512], fp32, tag="z")
                nc.vector.tensor_scalar_add(zt[:], ps[:], blg_sb[:, m:m + 1])
                sg = ptmp.tile([P, 512], fp32, tag="sg")
                nc.scalar.activation(sg[:], ps[:], AF.Sigmoid,
                                     bias=blg_sb[:, m:m + 1])
                lt = plat.tile([P, 512], bf16, tag="lat")
                nc.vector.tensor_mul(lt[:], zt[:], sg[:])
                nc.sync.dma_start(latg[P * m:P * (m + 1), ns], lt[:])


def _emit(tc, io):
    import concourse.bass as bass
    from concourse import mybir
    from concourse.masks import make_causal_mask, make_identity

    nc = tc.nc
    fp32 = mybir.dt.float32
    bf16 = mybir.dt.bfloat16
    AX = mybir.AxisListType
    AF = mybir.ActivationFunctionType

    xT, latT, wq, wk, wv, wq2kv, wkv2h, wo = (
        io["xT"], io["latT"], io["wq"], io["wk"], io["wv"],
        io["wq2kv"], io["wkv2h"], io["wo"],
    )
    bq, bk, bv, bkv2h = io["bq"], io["bk"], io["bv"], io["bkv2h"]
    out = io["out"]

    with ExitStack() as ctx:
        ek = ctx.enter_context

        # ---- long-lived pools -------------------------------------------
        pconst = ek(tc.tile_pool(name="const", bufs=1))
        pq2t = ek(tc.tile_pool(name="q2t", bufs=1))     # q2T per head [64, T]
        pkt = ek(tc.tile_pool(name="kt", bufs=1))       # kT [64, T]
        pv = ek(tc.tile_pool(name="v", bufs=1))         # v blocks [128, 65] x 16
        pc2t = ek(tc.tile_pool(name="c2t", bufs=1))     # ctx2T per head [128, T]

        # constants: shifted transposed causal masks M_d[r, c] = 0 where
        # c >= r + 128*d else -1e9, for diagonal block offsets d = 0..3.
        masks_t = []
        for d in range(4):
            mk = pconst.tile([P, 512], fp32, tag=f"mask{d}", name=f"mask{d}")
            nc.gpsimd.memset(mk[:], 0.0)
            nc.gpsimd.affine_select(
                out=mk[:], in_=mk[:], compare_op=mybir.AluOpType.is_ge,
                fill=-1e9, base=-P * d, pattern=[[1, 512]],
                channel_multiplier=-1)
            masks_t.append(mk)
        ones_row = pconst.tile([1, KV_DIM], bf16, tag="ones_row")
        nc.gpsimd.memset(ones_row[:], 1.0)
        bq_sb = pconst.tile([P, HPC], fp32, tag="bq")
        nc.sync.dma_start(bq_sb[:], bq[:])
        bk_sb = pconst.tile([KV_DIM, 1], fp32, tag="bk")
        nc.sync.dma_start(bk_sb[:], bk[:])
        bv_sb = pconst.tile([KV_DIM, 1], fp32, tag="bv")
        nc.sync.dma_start(bv_sb[:], bv[:])
        bkv2h_sb = pconst.tile([P, 1], fp32, tag="bkv2h")
        nc.sync.dma_start(bkv2h_sb[:], bkv2h[:])
        wq2kv_sb = pconst.tile([HEAD_DIM, KV_DIM], bf16, tag="wq2kv")
        nc.sync.dma_start(wq2kv_sb[:], wq2kv[:])
        wkv2h_sb = pconst.tile([KV_DIM, HEAD_DIM], bf16, tag="wkv2h")
        nc.sync.dma_start(wkv2h_sb[:], wkv2h[:])
        expb = pconst.tile([P, 1], fp32, tag="expb")
        nc.gpsimd.memset(expb[:], EXP_BIAS)

        q2t_sb = [pq2t.tile([KV_DIM, T], bf16, tag=f"q2t{h}", name=f"q2t{h}") for h in range(HPC)]
        kt_sb = pkt.tile([KV_DIM, T], bf16, tag="kt")
        # v_aug[j]: [128, 65] — col 64 is ones so attn@v also yields the
        # softmax denominator as row 64 of the (transposed) context.
        v_sb = [pv.tile([P, KV_DIM + 1], bf16, tag=f"v{j}", name=f"v{j}") for j in range(NB)]
        for j in range(NB):
            nc.gpsimd.memset(v_sb[j][:, KV_DIM:KV_DIM + 1], 1.0)
        c2t_sb = [pc2t.tile([P, T], bf16, tag=f"c2t{h}", name=f"c2t{h}") for h in range(HPC)]

        # ================= stage 1: projections ==========================
        with tc.tile_pool(name="s1w", bufs=1) as ps1w, \
             tc.tile_pool(name="s1x", bufs=24) as ps1x, \
             tc.tile_pool(name="s1q", bufs=10) as ps1q, \
             tc.tile_pool(name="s1lat", bufs=18) as ps1lat, \
             tc.tile_pool(name="s1tmp", bufs=3) as ps1tmp, \
             tc.tile_pool(name="s1ps", bufs=3, space="PSUM") as ps1ps:

            wq_sb = [ps1w.tile([P, QCOLS], bf16, tag=f"wq{k}", name=f"wqsb{k}") for k in range(NKT)]
            wk_sb = [ps1w.tile([P, KV_DIM], bf16, tag=f"wk{k}", name=f"wksb{k}") for k in range(LKT)]
            wv_sb = [ps1w.tile([P, KV_DIM], bf16, tag=f"wv{k}", name=f"wvsb{k}") for k in range(LKT)]
            for k in range(NKT):
                nc.sync.dma_start(wq_sb[k][:], wq[P * k:P * (k + 1), :])
            for k in range(LKT):
                nc.sync.dma_start(wk_sb[k][:], wk[P * k:P * (k + 1), :])
                nc.sync.dma_start(wv_sb[k][:], wv[P * k:P * (k + 1), :])

            for n in range(NQT):
                ns = slice(512 * n, 512 * (n + 1))
                x_n = []
                for k in range(NKT):
                    xt = ps1x.tile([P, 512], bf16, tag="x", name="xt")
                    nc.sync.dma_start(xt[:], xT[P * k:P * (k + 1), ns])
                    x_n.append(xt)

                # qT slices for the 4 heads (m = head), with bias, Identity
                q_n = []
                for m in range(HPC):
                    ps = ps1ps.tile([P, 512], fp32, tag="ps")
                    for k in range(NKT):
                        nc.tensor.matmul(
                            ps[:], wq_sb[k][:, P * m:P * (m + 1)], x_n[k][:],
                            start=(k == 0), stop=(k == NKT - 1))
                    qt = ps1q.tile([P, 512], bf16, tag="q")
                    nc.vector.tensor_scalar_add(qt[:], ps[:], bq_sb[:, m:m + 1])
                    q_n.append(qt)

                # latT slices come precomputed from launch A
                lat_n = []
                for lk in range(LKT):
                    lt = ps1lat.tile([P, 512], bf16, tag="lat", name="latB")
                    nc.sync.dma_start(lt[:], latT[P * lk:P * (lk + 1), ns])
                    lat_n.append(lt)

                # q2T for each head over this n-slice: [64, 512]
                for h in range(HPC):
                    ps = ps1ps.tile([P, 512], fp32, tag="ps")
                    nc.tensor.matmul(ps[:KV_DIM, :], wq2kv_sb[:], q_n[h][:],
                                     start=True, stop=True)
                    nc.vector.tensor_copy(q2t_sb[h][:, ns], ps[:KV_DIM, :])

                # kT over this n-slice: [64, 512] += over latent tiles
                ps = ps1ps.tile([P, 512], fp32, tag="ps")
                for lk in range(LKT):
                    nc.tensor.matmul(ps[:KV_DIM, :], wk_sb[lk][:], lat_n[lk][:],
                                     start=(lk == 0), stop=(lk == LKT - 1))
                nc.vector.tensor_scalar_add(kt_sb[:, ns], ps[:KV_DIM, :],
                                            bk_sb[:])

                # v blocks [128, 64] for the 4 kpos blocks in this n-slice
                for kb in range(4):
                    j = 4 * n + kb
                    bs = slice(P * kb, P * (kb + 1))
                    ps = ps1ps.tile([P, 512], fp32, tag="ps")
                    for lk in range(LKT):
                        nc.tensor.matmul(ps[:, :KV_DIM], lat_n[lk][:, bs],
                                         wv_sb[lk][:],
                                         start=(lk == 0), stop=(lk == LKT - 1))
                    nc.vector.tensor_copy(v_sb[j][:, :KV_DIM], ps[:, :KV_DIM])

        # ================= stage 2: attention (transposed probs) =========
        # scoresT[k, q] = kT_blk.T @ q2T — probs come out already transposed
        # for the attn@v matmul; v's ones-column makes row 64 of the context
        # PSUM the softmax denominator, applied afterwards via a K=1
        # broadcast matmul.
        with tc.tile_pool(name="s2pt", bufs=24) as ppt, \
             tc.tile_pool(name="s2small", bufs=3) as psmall, \
             tc.tile_pool(name="s2wo", bufs=1) as pwo, \
             tc.tile_pool(name="s2out", bufs=3) as pout, \
             tc.tile_pool(name="s2ps", bufs=3, space="PSUM") as pscore, \
             tc.tile_pool(name="s2ctx_ps", bufs=2, space="PSUM") as pctxps, \
             tc.tile_pool(name="s2bc_ps", bufs=2, space="PSUM") as pbcps:

            wo_sb = [pwo.tile([P, D_OUT], bf16, tag=f"wo{c}", name=f"wosb{c}") for c in range(HPC)]
            for c in range(HPC):
                nc.sync.dma_start(wo_sb[c][:], wo[P * c:P * (c + 1), :])

            for h in range(HPC):
                for n in range(NQT):
                    ns = slice(512 * n, 512 * (n + 1))
                    nj = 4 * n + 4       # causal: k-blocks 0 .. 4n+3
                    pts = []
                    for j in range(nj):
                        ps = pscore.tile([P, 512], fp32, tag="score")
                        nc.tensor.matmul(
                            ps[:], kt_sb[:, P * j:P * (j + 1)],
                            q2t_sb[h][:, ns], start=True, stop=True)
                        d = j - 4 * n
                        if d >= 0:
                            nc.vector.tensor_add(ps[:], ps[:], masks_t[d][:])
                        pt = ppt.tile([P, 512], bf16, tag="pt")
                        nc.scalar.activation(pt[:], ps[:], AF.Exp,
                                             bias=expb[:], scale=SCALE)
                        pts.append(pt)
                    pc = pctxps.tile([KV_DIM + 1, 512], fp32, tag="cx")
                    for j in range(nj):
                        nc.tensor.matmul(pc[:], v_sb[j][:], pts[j][:],
                                         start=(j == 0), stop=(j == nj - 1))
                    # denominator -> reciprocal -> broadcast over 64 rows
                    rec32 = psmall.tile([1, 512], fp32, tag="rec32")
                    nc.vector.reciprocal(rec32[:], pc[KV_DIM:KV_DIM + 1, :])
                    rec = psmall.tile([1, 512], bf16, tag="rec")
                    nc.vector.tensor_copy(rec[:], rec32[:])
                    bc = pbcps.tile([KV_DIM, 512], fp32, tag="bc")
                    nc.tensor.matmul(bc[:], ones_row[:], rec[:],
                                     start=True, stop=True)
                    bcs = psmall.tile([KV_DIM, 512], fp32, tag="bcs")
                    nc.vector.tensor_copy(bcs[:], bc[:])
                    ctxn = psmall.tile([KV_DIM, 512], bf16, tag="ctxn")
                    nc.vector.tensor_mul(ctxn[:], pc[:KV_DIM, :], bcs[:])
                    # kv2h; bias holds bkv2h + Wkv2h.T @ bv (host-folded)
                    ps2 = pscore.tile([P, 512], fp32, tag="score")
                    nc.tensor.matmul(ps2[:], wkv2h_sb[:], ctxn[:],
                                     start=True, stop=True)
                    nc.vector.tensor_scalar_add(c2t_sb[h][:, ns], ps2[:],
                                                bkv2h_sb[:])

            # ============= stage 3: output projection (partial) ==========
            for qb in range(NB):
                qs = slice(P * qb, P * (qb + 1))
                osb = pout.tile([P, D_OUT], fp32, tag="osb")
                for ot in range(4):
                    ops = slice(512 * ot, 512 * (ot + 1))
                    ps = pscore.tile([P, 512], fp32, tag="score")
                    for c in range(HPC):
                        nc.tensor.matmul(ps[:], c2t_sb[c][:, qs],
                                         wo_sb[c][:, ops],
                                         start=(c == 0), stop=(c == HPC - 1))
                    nc.vector.tensor_copy(osb[:, ops], ps[:])
                nc.sync.dma_start(out[qs, :], osb[:])


def _build_program_a():
    import concourse.tile as tile
    from concourse import bacc, mybir

    nc = bacc.Bacc("TRN2", target_bir_lowering=False, debug=False,
                   enable_asserts=False, num_devices=8)
    f32 = mybir.dt.float32
    bf16 = mybir.dt.bfloat16

    def din(name, shape, dt):
        return nc.dram_tensor(name, shape, dt, kind="ExternalInput").ap()

    io = {
        "xT": din("xT", [D_IN, T], bf16),
        "wlg": din("wlg", [D_IN, LATENT // 4], bf16),
        "blg": din("blg", [P, LATENT // 4 // P], f32),
        "latg": nc.dram_tensor("latg", [LATENT // 4, T], bf16,
                               kind="ExternalOutput").ap(),
    }
    with tile.TileContext(nc) as tc:
        _emit_lat(tc, io)
    nc.compile()
    return nc


def _build_program_b():
    import concourse.tile as tile
    from concourse import bacc, mybir

    nc = bacc.Bacc("TRN2", target_bir_lowering=False, debug=False,
                   enable_asserts=False, num_devices=8)
    f32 = mybir.dt.float32
    bf16 = mybir.dt.bfloat16

    def din(name, shape, dt):
        return nc.dram_tensor(name, shape, dt, kind="ExternalInput").ap()

    io = {
        "xT": din("xT", [D_IN, T], bf16),
        "latT": din("latT", [LATENT, T], bf16),
        "wq": din("wq", [D_IN, QCOLS], bf16),
        "wk": din("wk", [LATENT, KV_DIM], bf16),
        "wv": din("wv", [LATENT, KV_DIM], bf16),
        "wq2kv": din("wq2kv", [HEAD_DIM, KV_DIM], bf16),
        "wkv2h": din("wkv2h", [KV_DIM, HEAD_DIM], bf16),
        "wo": din("wo", [QCOLS, D_OUT], bf16),
        "bq": din("bq", [P, HPC], f32),
        "bk": din("bk", [KV_DIM, 1], f32),
        "bv": din("bv", [KV_DIM, 1], f32),
        "bkv2h": din("bkv2h", [P, 1], f32),
        "out": nc.dram_tensor("out", [T, D_OUT], f32, kind="ExternalOutput").ap(),
    }
    with tile.TileContext(nc) as tc:
        _emit(tc, io)
    nc.compile()
    return nc


def _get_program(which="b"):
    key = f"nc_{which}"
    if key not in _PROGRAM_CACHE:
        _PROGRAM_CACHE[key] = (
            _build_program_a() if which == "a" else _build_program_b())
    return _PROGRAM_CACHE[key]


def make_xt(inputs):
    x = np.asarray(inputs["x"], np.float32)
    return [np.ascontiguousarray(x[b].T).astype(BF16) for b in range(B)]


def make_in_maps_a(inputs, xT_b):
    Wl = np.asarray(inputs["Wl"], np.float32)
    bl = np.asarray(inputs["bl"], np.float32)
    LG = LATENT // 4
    in_maps = []
    for core in range(8):
        b, g = core // 4, core % 4
        ls = slice(LG * g, LG * (g + 1))
        in_maps.append({
            "xT": xT_b[b],
            "wlg": np.ascontiguousarray(Wl[:, ls]).astype(BF16),
            "blg": np.ascontiguousarray(bl[ls].reshape(LG // P, P).T),
        })
    return in_maps


def gather_lat(results_a):
    """Concat the 4 per-core latent slices into latT per batch."""
    return [np.concatenate(
        [np.asarray(results_a[4 * b + g]["latg"]) for g in range(4)], axis=0)
        for b in range(B)]


def make_in_maps_b(inputs, xT_b, latT_b):
    Wq = np.asarray(inputs["Wq"], np.float32)
    Wk = np.asarray(inputs["Wk"], np.float32)
    Wv = np.asarray(inputs["Wv"], np.float32)
    Wq2kv = np.asarray(inputs["Wq2kv"], np.float32)
    Wkv2h = np.asarray(inputs["Wkv2h"], np.float32)
    Wo = np.asarray(inputs["Wo"], np.float32)
    bq = np.asarray(inputs["bq"], np.float32)
    bk = np.asarray(inputs["bk"], np.float32)
    bv = np.asarray(inputs["bv"], np.float32)
    bkv2h = np.asarray(inputs["bkv2h"], np.float32)

    wq2kv_b = np.ascontiguousarray(Wq2kv).astype(BF16)
    wkv2h_b = np.ascontiguousarray(Wkv2h).astype(BF16)

    in_maps = []
    for core in range(8):
        b, g = core // 4, core % 4
        cs = slice(QCOLS * g, QCOLS * (g + 1))
        ks = slice(KV_DIM * g, KV_DIM * (g + 1))
        in_maps.append({
            "xT": xT_b[b],
            "latT": latT_b[b],
            "wq": np.ascontiguousarray(Wq[:, cs]).astype(BF16),
            "wk": np.ascontiguousarray(Wk[:, ks]).astype(BF16),
            "wv": np.ascontiguousarray(Wv[:, ks]).astype(BF16),
            "wq2kv": wq2kv_b,
            "wkv2h": wkv2h_b,
            "wo": np.ascontiguousarray(Wo[cs, :]).astype(BF16),
            "bq": np.ascontiguousarray(bq[cs].reshape(HPC, P).T),
            "bk": np.ascontiguousarray(bk[ks].reshape(KV_DIM, 1)),
            "bv": np.ascontiguousarray(bv[ks].reshape(KV_DIM, 1)),
            # bv folded into the kv2h bias: p@(v+bv) @ Wkv2h + bkv2h
            #   == p@v @ Wkv2h + (Wkv2h.T @ bv[ks] + bkv2h)  (rows sum to 1)
            "bkv2h": (bkv2h + Wkv2h.T @ bv[ks]).reshape(P, 1),
        })
    return in_maps


def assemble(inputs, results):
    bo = np.asarray(inputs["bo"], np.float32)
    y = np.zeros((B, T, D_OUT), np.float32)
    for core in range(8):
        b = core // 4
        y[b] += np.asarray(results[core]["out"], np.float32)
    y += bo[None, None, :]
    return y


def kernel(**inputs):
    from concourse.bass_utils import run_bass_kernel_spmd
    nca = _get_program("a")
    ncb = _get_program("b")
    xT_b = make_xt(inputs)
    res_a = run_bass_kernel_spmd(nca, make_in_maps_a(inputs, xT_b),
                                 core_ids=list(range(8)))
    latT_b = gather_lat(res_a.results)
    res_b = run_bass_kernel_spmd(ncb, make_in_maps_b(inputs, xT_b, latT_b),
                                 core_ids=list(range(8)))
    return assemble(inputs, res_b.results)

